# revision 14
# baseline (speedup 1.0000x reference)
"""Multi-head causal attention (B=1024, T=64, C=768, H=12, D=64) on 8 TRN2
NeuronCores, data-parallel over the batch dimension (128 batches/core).

v2 dataflow per core (1024-token chunks, TT=8 128-token tiles, NB=16 batches):
  - All transposes ride the XBAR DMA engine (dma_start_transpose, 14ns per
    16x128 tile): X tiles, Y tiles and all four weight preps. The PE runs
    only "real" matmuls.
  - Q/K projections run in fp8(e4m3) DoubleRow perf mode (2 contraction
    rows/partition/cycle): W is pre-scaled by 2^10 into fp8, X is cast
    straight to fp8 (validated: final rel err ~1.6e-2 < 2e-2 gate); the
    2^-20 descale is folded into the softmax exp scale. V / output
    projections stay bf16.
  - scores+exp and the V projection interleave per tile so the Scalar exp
    drains while the PE streams V; AV / normalize / XBAR-Y / output
    projection software-pipeline per tile as before.
  - Evacuations are spread across engines: q+mask+normalize+recip+x-fp8
    casts on DVE, k+v_sb+exp on Scalar, out-bias add on GpSimd, XBARs on
    the SP/Activation HWDGE queues.
"""

import numpy as np

P = 128
B, T, C, H, Dh = 1024, 64, 768, 12, 64
HD = H * Dh            # 768
NCC = C // P           # 6 contraction chunks
NHD = HD // P          # 6 hd chunks
N_CORES = 8
F8CC = 3               # how many 256-channel pairs of the QK contraction run fp8
                       # (3 = all 768 channels; 2 = 512 fp8 + 256 bf16 fallback)

_cache = {}


def _patch_tile_drain(tile, mybir):
    """walrus CTRL (Drain) ops in this toolchain accept only 1 sem-wait;
    spread the TileContext exit-drain's waits across preceding SP nops."""
    from concourse.vector_clock import ScopedClock

    if getattr(tile.TileContext, "_drain_patched", False):
        return

    def _drain_and_barrier(self, tick_clock, wait_clock):
        nc = self.nc
        drain_inst = nc.sync.drain()
        wait_clock.add_sem_waits(
            drain_inst.ins, ScopedClock({None: tick_clock.global_clock})
        )
        waits = list(drain_inst.ins.sync_info.on_wait)
        if len(waits) > 1:
            drain_inst.ins.sync_info.on_wait = waits[-1:]
            cur_bb = nc.cur_bb.bb
            idx = cur_bb.instructions.index(drain_inst.ins)
            extra = []
            for w in waits[:-1]:
                nop = mybir.InstNoOp(name=f"I-{nc.next_id()}", ins=[], outs=[])
                nop.engine = drain_inst.ins.engine
                nop.sync_info = mybir.SyncInfo(on_wait=[w], on_update=[])
                nc.register_instruction(nop)
                extra.append(nop)
            cur_bb.instructions[idx:idx] = extra
        nc.all_engine_barrier()
        assert self.sems is not None
        popped = nc._tile_sem_poison_stack.pop()
        assert popped is self._sem_poison
        nc.clear_and_free_semaphores(list(self.sems.allocated().values()))
        nc.all_engine_barrier()

    tile.TileContext._drain_and_barrier = _drain_and_barrier
    tile.TileContext._drain_patched = True


def _install_loud_cc_hook():
    """Surface real exceptions from the neuronx_cc hook (C wrapper eats them)."""
    from concourse import bass2jax as _b2j
    if getattr(_b2j, "_loud_hook_installed", False):
        return
    _orig = _b2j.neuronx_cc_hook
    def _loud(*a, **k):
        try:
            return _orig(*a, **k)
        except BaseException:
            import traceback
            traceback.print_exc()
            raise
    _b2j.neuronx_cc_hook = _loud
    _b2j._loud_hook_installed = True


def _split_multi_waits(nc, mybir, K=1):
    """This walrus build supports only one sem-wait per instruction: move
    excess waits onto same-engine NOPs inserted directly before the owner."""
    def fix_block(bb):
        insts = bb.instructions
        i = 0
        while i < len(insts):
            ins = insts[i]
            si = ins.sync_info
            w = list(si.on_wait) if si is not None and si.on_wait else []
            if len(w) > K:
                carriers = []
                for j in range(0, len(w) - K, K):
                    nop = mybir.InstNoOp(name=f"I-{nc.next_id()}", ins=[], outs=[])
                    nop.engine = ins.engine
                    nop.sync_info = mybir.SyncInfo(on_wait=w[j:j + K], on_update=[])
                    nc.register_instruction(nop)
                    carriers.append(nop)
                si.on_wait = w[len(w) - K:]
                insts[i:i] = carriers
                i += len(carriers)
            i += 1
    for fn in nc.m.functions:
        for bb in fn.blocks:
            fix_block(bb)


def _bp_bcast_ap(bass, bp_d):
    a = bp_d[:]
    return bass.AP(tensor=a.tensor, offset=a.offset, ap=[[0, P]] + list(a.ap))


def build_nc(B_loc=B // N_CORES, chunk_tok=1024):
    import concourse.bass as bass
    import concourse.tile as tile
    from concourse import mybir
    from contextlib import ExitStack

    _patch_tile_drain(tile, mybir)
    _install_loud_cc_hook()

    F32 = mybir.dt.float32
    BF16 = mybir.dt.bfloat16
    F8 = mybir.dt.float8e4
    AF = mybir.ActivationFunctionType
    ALU = mybir.AluOpType
    DR = mybir.MatmulPerfMode.DoubleRow

    BT = B_loc * T
    chunk_tok = min(chunk_tok, BT)
    n_chunks = BT // chunk_tok
    assert n_chunks * chunk_tok == BT
    TT = chunk_tok // P     # 128-token tiles per chunk
    NB = chunk_tok // T     # batches per chunk

    W8SC = 1024.0           # fp8 weight pre-scale (2^10)
    EXPSC = 0.125 * (2.0 ** -20) if F8CC > 0 else 0.125
    # the bf16 contraction tail would accumulate unscaled q/k into the
    # 2^10-scaled fp8 PSUM; scale its weights too if ever enabled
    assert F8CC in (0, 3), "mixed-precision tail needs scaled bf16 weights"

    nc = bass.Bass()
    x_d = nc.declare_dram_parameter("x", [B_loc, T, C], F32, isOutput=False)
    wq_d = nc.declare_dram_parameter("Wq", [H, Dh, C], F32, isOutput=False)
    wk_d = nc.declare_dram_parameter("Wk", [H, Dh, C], F32, isOutput=False)
    wv_d = nc.declare_dram_parameter("Wv", [H, Dh, C], F32, isOutput=False)
    wp_d = nc.declare_dram_parameter("Wp", [C, HD], F32, isOutput=False)
    bp_d = nc.declare_dram_parameter("bp", [C], F32, isOutput=False)
    mk_d = nc.declare_dram_parameter("mask", [P, 2 * NHD * T], BF16, isOutput=False)
    out_d = nc.declare_dram_parameter("out", [B_loc, T, C], F32, isOutput=True)

    xf = x_d[:].flatten_outer_dims()      # [BT, C]
    of = out_d[:].flatten_outer_dims()    # [BT, C]

    with tile.TileContext(nc) as tc, ExitStack() as ctx:
        sing = ctx.enter_context(tc.tile_pool(name="sing", bufs=1))
        w4_p = ctx.enter_context(tc.tile_pool(name="w4", bufs=2 if F8CC == 3 else 4))
        wbf_p = ctx.enter_context(tc.tile_pool(name="wbfp", bufs=8))
        xbf_p = ctx.enter_context(tc.tile_pool(name="xbfp", bufs=6))
        xT_p = ctx.enter_context(tc.tile_pool(name="xTp", bufs=2))
        x8_p = ctx.enter_context(tc.tile_pool(name="x8p", bufs=2))
        qT_p = ctx.enter_context(tc.tile_pool(name="qTp", bufs=1))
        vsb_p = ctx.enter_context(tc.tile_pool(name="vsb", bufs=3))
        pex_p = ctx.enter_context(tc.tile_pool(name="pex", bufs=1))
        y_p = ctx.enter_context(tc.tile_pool(name="y", bufs=4))
        yt_p = ctx.enter_context(tc.tile_pool(name="yt", bufs=6))
        ostage = ctx.enter_context(tc.tile_pool(name="ostage", bufs=3))
        small = ctx.enter_context(tc.tile_pool(name="small", bufs=6))
        pp = ctx.enter_context(tc.tile_pool(name="pp", bufs=8, space="PSUM"))

        def ptile(pdim, shape, name, dt=None):
            # all PSUM tiles share one 1-bank slot class for max in-flight tiles
            t = pp.tile([P, 512], dt or F32, tag="ps", name=name)
            flat = t[:pdim, : int(np.prod(shape[1:]))]
            return flat.rearrange(
                "p (a b) -> p a b", a=shape[1]
            ) if len(shape) == 3 else flat

        # ---- weight loads first: these DMAs gate the first chunk ----
        # each W: 6 casting DMAs (f32 DRAM -> bf16 SBUF) + 6 XBAR transposes
        # wT4[p, r, b, s] = Wflat[r*128 + s, b*128 + p]
        wflats = {
            "wk": wk_d[:].flatten_outer_dims(),
            "wq": wq_d[:].flatten_outer_dims(),
            "wv": wv_d[:].flatten_outer_dims(),
            "wp": wp_d[:],
        }
        # ---- weight load + XBAR transpose (load on SWDGE, XBAR on scalar) ----
        # wT4[p, r, b, s] = Wflat[r*128 + s, b*128 + p]
        def prep_w4(wname, pool):
            wT4 = pool.tile([P, 6, NCC, P], BF16, name=f"{wname}T4", tag="w4")
            for r in range(6):
                wbf = wbf_p.tile([P, C], BF16, tag="wbf", name=f"{wname}_bf{r}")
                nc.gpsimd.dma_start(out=wbf, in_=wflats[wname][r * P:(r + 1) * P, :])
                nc.scalar.dma_start_transpose(out=wT4[:, r, :, :], in_=wbf)
            return wT4

        # fp8 copies of wk/wq (x W8SC), on DVE, sliced per m for pipelining
        def cast_w8(wT4, name):
            w8 = sing.tile([P, 6, NCC, P], F8, name=name)
            for m in range(6):
                nc.vector.tensor_scalar_mul(w8[:, m], wT4[:, m], W8SC)
            return w8

        # ---- x loads (casting DMAs on the SWDGE queue) ----
        def p0a(ci):
            tok0 = ci * chunk_tok
            xbs = []
            for it in range(TT):
                xb = xbf_p.tile([P, C], BF16, tag="xbf")
                nc.gpsimd.dma_start(out=xb, in_=xf[tok0 + it * P:tok0 + (it + 1) * P, :])
                xbs.append(xb)
            return xbs

        # priority order: first chunk's QK needs wk/wq + x; wv/wp trail
        wkT4 = prep_w4("wk", w4_p)
        wqT4 = prep_w4("wq", w4_p)
        wk8 = cast_w8(wkT4, "wk8")
        wq8 = cast_w8(wqT4, "wq8")
        xbs_cur = p0a(0)

        mask_sb = sing.tile([P, 2, NHD, T], BF16)
        nc.sync.dma_start(out=mask_sb, in_=mk_d[:].rearrange(
            "p (two a b) -> p two a b", two=2, a=NHD))

        # ktbd zeros must land before chunk 0's K evacuation
        ktbd = sing.tile([P, NHD, NB, P], BF16, name="ktbd")
        nc.vector.memset(ktbd, 0.0)

        # wv/wp reuse the wk/wq bf16 buffers once the fp8 casts are done
        wvT4 = prep_w4("wv", w4_p)
        wpT4 = prep_w4("wp", w4_p)
        bp_bc = sing.tile([P, C], F32)
        nc.gpsimd.dma_start(out=bp_bc, in_=_bp_bcast_ap(bass, bp_d))
        # rhs views: [p, cc, m, s] ordering for V and O projections
        wvT4r = wvT4.rearrange("p m c s -> p c m s")
        wpT4r = wpT4.rearrange("p r j c -> p j r c")

        # ---- persistent block-diagonal V operand ----
        vbd = sing.tile([P, NHD, NB, 2 * (Dh + 1)], BF16, name="vbd")
        nc.vector.memset(vbd, 0.0)
        nc.vector.memset(vbd[0:T, :, :, Dh:Dh + 1], 1.0)
        nc.vector.memset(vbd[T:P, :, :, 2 * Dh + 1:2 * Dh + 2], 1.0)

        # ---- per-chunk X pipeline: XBAR transpose + fp8 cast ----
        # x8 is cc-major [c-part, cc, tok] so each QK psum bank accumulates
        # in ONE start/stop group with 512-wide streams
        def p0b(xbs):
            xT = xT_p.tile([P, TT, NCC, P], BF16, tag="xT")
            x8 = x8_p.tile([P, NCC, chunk_tok], F8, tag="x8")
            for it in range(TT):
                nc.sync.dma_start_transpose(out=xT[:, it, :, :], in_=xbs[it])
            for it in range(TT):
                nc.vector.tensor_copy(
                    out=x8[:, :, it * P:(it + 1) * P], in_=xT[:, it])
            return xT, x8

        xT, x8 = p0b(xbs_cur)

        for ci in range(n_chunks):
            tok0 = ci * chunk_tok

            # ---- P1a: Q/K projections, fp8 DoubleRow (+ bf16 tail) ----
            qT = qT_p.tile([P, NHD, chunk_tok], BF16, tag="qT")
            nbsub = 512 // T
            for w8t, wT4t, dst in ((wk8, wkT4, "k"), (wq8, wqT4, "q")):
                for m in range(NHD):
                    for s in range(TT // 4):
                        psf = ptile(P, (P, 512), f"qk_{dst}{m}{s}")
                        for cc in range(F8CC):
                            nc.tensor.matmul(
                                psf, w8t[:, m, 2 * cc:2 * cc + 2, :],
                                x8[:, 2 * cc:2 * cc + 2, s * 512:(s + 1) * 512],
                                start=(cc == 0), stop=(cc == F8CC - 1),
                                perf_mode=DR)
                        if dst == "q":
                            nc.vector.tensor_copy(
                                out=qT[:, m, s * 512:(s + 1) * 512],
                                in_=psf)
                        else:
                            b0 = s * nbsub
                            nc.scalar.copy(
                                out=ktbd[0:T, m, b0:b0 + nbsub, 0:T],
                                in_=psf[0:T].rearrange(
                                    "p (nb t) -> p nb t", nb=nbsub))
                            nc.scalar.copy(
                                out=ktbd[T:P, m, b0:b0 + nbsub, T:P],
                                in_=psf[T:P].rearrange(
                                    "p (nb t) -> p nb t", nb=nbsub))

            # ---- P2a+P1b interleaved per tile: scores+exp+mask | V proj ----
            pex_all = pex_p.tile([P, NB, NHD, T], BF16, tag="pex")
            vbd_v = vbd.rearrange("p a (nb2 two) c -> p a nb2 two c", two=2)
            for it in range(TT):
                for half in range(2):
                    b = 2 * it + half
                    s_ps = ptile(P, (P, NHD, T), f"s_ps{b % 2}")
                    for m in range(NHD):
                        nc.tensor.matmul(
                            s_ps[:, m, :], ktbd[:, m, b, :],
                            qT[:, m, b * T:(b + 1) * T],
                            start=True, stop=True)
                    nc.scalar.activation(
                        out=pex_all[:, b], in_=s_ps, func=AF.Exp, scale=EXPSC)
                nc.vector.tensor_tensor(
                    pex_all[:, 2 * it:2 * it + 2], pex_all[:, 2 * it:2 * it + 2],
                    mask_sb, ALU.mult)
                # V projection for this tile
                psA = ptile(P, (P, 512), "v_psA")
                psB = ptile(P, (P, 256), "v_psB")
                for cc in range(NCC):
                    lhs = xT[:, it, cc, :]
                    nc.tensor.matmul(psA, lhs, wvT4r[:, cc, 0:4, :],
                                     start=(cc == 0), stop=(cc == NCC - 1))
                    nc.tensor.matmul(psB, lhs, wvT4r[:, cc, 4:6, :],
                                     start=(cc == 0), stop=(cc == NCC - 1))
                v_sb = vsb_p.tile([P, H, Dh], BF16, tag="v_sb")
                nc.scalar.copy(
                    out=v_sb[:, 0:8, :], in_=psA.rearrange("p (a b) -> p a b", a=8))
                nc.scalar.copy(
                    out=v_sb[:, 8:12, :], in_=psB.rearrange("p (a b) -> p a b", a=4))
                v_sb2 = v_sb.rearrange("p (h two) c -> p h two c", two=2)
                for par in range(2):
                    nc.gpsimd.dma_start(
                        out=vbd_v[0:T, :, it, par, 0:Dh],
                        in_=v_sb2[par * T:(par + 1) * T, :, 0, :])
                    nc.gpsimd.dma_start(
                        out=vbd_v[T:P, :, it, par, Dh + 1:2 * Dh + 1],
                        in_=v_sb2[par * T:(par + 1) * T, :, 1, :])

            # ---- next chunk X loads (cheap SWDGE casting DMAs) ----
            if ci + 1 < n_chunks:
                xbs_next = p0a(ci + 1)

            # ---- P2b/P3: AV + normalize + XBAR-Y + output projection ----
            ybs = []
            ytiles = []
            def yt_tr(yb):
                ytile = yt_p.tile([P, NHD, P], BF16, tag="ytile")
                nc.sync.dma_start_transpose(out=ytile, in_=yb)
                ytiles.append(ytile)
            def oproj_emit(it):
                ytile = ytiles[it]
                oA = ptile(P, (P, 512), "o_psA")
                oB = ptile(P, (P, 256), "o_psB")
                for j in range(NHD):
                    lhs = ytile[:, j, :]
                    nc.tensor.matmul(oA, lhs, wpT4r[:, j, 0:4, :],
                                     start=(j == 0), stop=(j == NHD - 1))
                    nc.tensor.matmul(oB, lhs, wpT4r[:, j, 4:6, :],
                                     start=(j == 0), stop=(j == NHD - 1))
                osb = ostage.tile([P, C], F32, tag="osb")
                nc.vector.tensor_tensor(osb[:, 0:512], oA, bp_bc[:, 0:512], ALU.add)
                nc.vector.tensor_tensor(osb[:, 512:768], oB, bp_bc[:, 512:768], ALU.add)
                row0 = tok0 + it * P
                nc.sync.dma_start(out=of[row0:row0 + P, :], in_=osb)

            for it in range(TT):
                yb = y_p.tile([P, HD], BF16, tag="yb")
                ybs.append(yb)
                y_ps = [ptile(P, (P, 3, 2 * (Dh + 1)), f"y_ps{h2}") for h2 in range(2)]
                for m in range(NHD):
                    for half in range(2):
                        b = it * 2 + half
                        prow = half * T
                        nc.tensor.matmul(
                            y_ps[m // 3][prow:prow + T, m % 3, :],
                            pex_all[:, b, m, :],
                            vbd[:, m, b, :],
                            start=True, stop=True)
                for h2 in range(2):
                    y_v = y_ps[h2].rearrange("p a (two c) -> p a two c", c=Dh + 1)
                    rec = small.tile([P, 3, 2, 1], F32, tag="rec", name="rec")
                    nc.vector.reciprocal(out=rec, in_=y_v[:, :, :, Dh:Dh + 1])
                    nc.vector.tensor_tensor(
                        yb[:, h2 * 384:(h2 + 1) * 384]
                            .rearrange("p (a two b) -> p a two b", a=3, two=2),
                        y_v[:, :, :, 0:Dh],
                        rec.to_broadcast([P, 3, 2, Dh]),
                        ALU.mult)
                if it >= 2:
                    yt_tr(ybs[it - 2])
                if it >= 3:
                    oproj_emit(it - 3)
            yt_tr(ybs[TT - 2])
            oproj_emit(TT - 3)
            yt_tr(ybs[TT - 1])
            oproj_emit(TT - 2)
            if ci + 1 < n_chunks:
                xT_next, x8_next = p0b(xbs_next)
            oproj_emit(TT - 1)
            if ci + 1 < n_chunks:
                xT, x8 = xT_next, x8_next

    _split_multi_waits(nc, mybir)
    return nc


def _get_program(B_loc, chunk_tok):
    key = (B_loc, chunk_tok)
    if key not in _cache:
        _cache[key] = build_nc(B_loc, chunk_tok)
    return _cache[key]


def make_const_inputs():
    import ml_dtypes
    # mask[s, t] = 1 if s <= t (causal, scoresT layout)
    m = np.tril(np.ones((T, T), dtype=np.float32)).T
    m2 = np.vstack([m, m])   # replicated for both partition-halves
    mask = np.tile(m2, (1, 2 * NHD)).astype(ml_dtypes.bfloat16)  # [P, 2*NHD*T]
    return mask


def prepare(x, Wq, Wk, Wv, Wp, bp, chunk_tok=1024):
    x = np.ascontiguousarray(x, dtype=np.float32)
    B_loc = B // N_CORES
    mask = make_const_inputs()
    nc = _get_program(B_loc, chunk_tok)
    in_maps = []
    for c in range(N_CORES):
        in_maps.append({
            "x": x[c * B_loc:(c + 1) * B_loc],
            "Wq": np.ascontiguousarray(Wq, dtype=np.float32),
            "Wk": np.ascontiguousarray(Wk, dtype=np.float32),
            "Wv": np.ascontiguousarray(Wv, dtype=np.float32),
            "Wp": np.ascontiguousarray(Wp, dtype=np.float32),
            "bp": np.ascontiguousarray(bp, dtype=np.float32),
            "mask": mask,
        })
    return nc, in_maps


def kernel(x, Wq, Wk, Wv, Wp, bp):
    from concourse import bass_utils

    nc, in_maps = prepare(x, Wq, Wk, Wv, Wp, bp)
    res = bass_utils.run_bass_kernel_spmd(nc, in_maps, list(range(N_CORES)))
    return np.concatenate([res.results[c]["out"] for c in range(N_CORES)], axis=0)
